# revision 10
# baseline (speedup 1.0000x reference)
"""Differential multi-head self-attention on 8 Trainium2 NeuronCores.

Sharding: core c handles batch b = c // 4 and heads {2*(c%4), 2*(c%4)+1}
(data parallel over batch, tensor parallel over heads). One SPMD Bass
program runs on all 8 cores; every per-core difference flows through the
input data. Each core emits per-head partial output projections
(o_h @ Wo_h, with rms_w and the (1 - lambda_init) factor folded into Wo
on the host); the host sums the partials per batch and adds bo.

Device math per (b, h):
  xT = x.T (PE transpose), QT/KT = W.T @ xT, V = x @ Wv   (f32r matmuls)
  per q tile of 256, k-chunk-major fused loop:
    St_half[k, q] = K_half @ Q_half.T  (f32r, causal tiles only)
    E_half = exp(St / sqrt(half)) * keep_mask   (ACT exp, bf16 output)
    O_half[q, d] += E_half[kc].T @ V[kc]        (bf16 matmuls, f32 PSUM)
    sums_half[q] += E_half[kc].T @ ones         (N=1 matmuls)
  O = O1/sums1 - lam * O2/sums2       (DVE per-partition scalars)
  r = exp(-0.5 * ln(eps + mean(O^2))) (ACT Square/Ln/Exp, single table set)
  out_h[s, e] = r[s] * (O.T @ Wo_h')[s, e]  (PE transpose + f32r matmul,
                                             r applied on the PSUM drain)
"""

import numpy as np
import ml_dtypes

import concourse.bass as bass
import concourse.mybir as mybir
import concourse.tile as tile
from concourse import bacc
from concourse.bass_utils import run_bass_kernel_spmd
from concourse.hw_specs import get_activation_tables
from concourse.masks import make_identity

B, S, E, H, D = 2, 2048, 512, 8, 512
HALF = D // 2
HLOC = 2            # heads per core
NCORES = 8
QT = 256            # q tile (free dim of score matmuls)
NQT = S // QT       # 8
KC = 128            # k chunk (partition dim of score tiles)
NKC = S // KC       # 16
NQC = QT // 128     # 2 q chunks per q tile
NDC = D // 128      # 4
NEC = E // 128      # 4
NSC = S // 128      # 16
KT_TILE = 512
SCALE = 1.0 / float(np.sqrt(HALF))
EPS = float(np.finfo(np.float32).eps)
LAMBDA_INIT = 0.8

f32 = mybir.dt.float32
f32r = mybir.dt.float32r
bf16 = mybir.dt.bfloat16
AF = mybir.ActivationFunctionType
ALU = mybir.AluOpType

SKIP, FULL = -1, -2


def _analyze_mask(mask):
    """Per (q-tile, k-chunk) block status: SKIP / FULL / keep-pattern index."""
    status = [[SKIP] * NKC for _ in range(NQT)]
    pats = []
    pat_idx = {}
    for t in range(NQT):
        for kc in range(NKC):
            blk = mask[t * QT:(t + 1) * QT, kc * KC:(kc + 1) * KC]  # [256 q, 128 k]
            if blk.all():
                status[t][kc] = SKIP
            elif not blk.any():
                status[t][kc] = FULL
            else:
                keep = (~blk).T.astype(np.float32)  # [128 k, 256 q]
                key = keep.tobytes()
                if key not in pat_idx:
                    pat_idx[key] = len(pats)
                    pats.append(keep)
                status[t][kc] = pat_idx[key]
    return status, pats


def _build(status, npat, repeat=1):  # noqa: C901
    nc = bacc.Bacc("TRN2", target_bir_lowering=False, debug=False)

    x_d = nc.dram_tensor("x", [S, E], f32, kind="ExternalInput")
    wq_d = nc.dram_tensor("wq", [HLOC, E, D], f32, kind="ExternalInput")
    wk_d = nc.dram_tensor("wk", [HLOC, E, D], f32, kind="ExternalInput")
    wv_d = nc.dram_tensor("wv", [HLOC, E, D], f32, kind="ExternalInput")
    wo_d = nc.dram_tensor("wo", [HLOC, D, E], f32, kind="ExternalInput")
    lamneg_d = nc.dram_tensor("lamneg", [HLOC, 128, 1], f32, kind="ExternalInput")
    keeps_d = nc.dram_tensor("keeps", [npat, 128, QT], bf16, kind="ExternalInput")
    out_d = nc.dram_tensor("out", [HLOC, S, E], f32, kind="ExternalOutput")
    iters_d = nc.dram_tensor("iters", [1, 1], f32, kind="ExternalOutput") if repeat > 1 else None

    act_sets = list(get_activation_tables(nc.m.arch).keys())
    nle_set = act_sets.index("natural_log_exp_and_others")

    with tile.TileContext(nc) as tc:
        with tc.tile_pool(name="cst", bufs=1) as cst, \
             tc.tile_pool(name="big", bufs=1) as big, \
             tc.tile_pool(name="epool", bufs=2) as epool, \
             tc.tile_pool(name="wts", bufs=1) as wts, \
             tc.tile_pool(name="qtp", bufs=2) as qtp, \
             tc.tile_pool(name="otp", bufs=2) as otp, \
             tc.tile_pool(name="scr", bufs=2) as scr, \
             tc.tile_pool(name="ps", bufs=3, space="PSUM") as ps, \
             tc.tile_pool(name="pso", bufs=4, space="PSUM") as pso, \
             tc.tile_pool(name="pss", bufs=1, space="PSUM") as pss:

            # One ACT table set covers Exp/Ln/Square: load it once up front.
            nc.scalar.add_instruction(mybir.InstLoadActFuncSet(
                name=nc.get_next_instruction_name(),
                ins=[], outs=[], act_func_set_id=nle_set))

            ident = cst.tile([128, 128], f32, tag="ident")
            make_identity(nc, ident[:])
            ones_bf = cst.tile([128, 1], bf16, tag="ones")
            nc.gpsimd.memset(ones_bf[:], 1.0)
            keeps_t = cst.tile([128, max(npat, 1), QT], bf16, tag="keeps")
            for i in range(npat):
                nc.sync.dma_start(keeps_t[:, i, :], keeps_d.ap()[i])
            lam_t = cst.tile([128, HLOC], f32, tag="lam")
            eps_t = cst.tile([128, 1], f32, tag="eps")
            nc.gpsimd.memset(eps_t[:], EPS)
            for h in range(HLOC):
                nc.sync.dma_start(lam_t[:, h:h + 1], lamneg_d.ap()[h])

            if repeat > 1:
                ctr = cst.tile([1, 1], f32, tag="ctr")
                nc.gpsimd.memset(ctr[:], 0.0)
            rep_ctx = tc.For_i(0, repeat, 1) if repeat > 1 else None
            if rep_ctx is not None:
                rep_ctx.__enter__()
                nc.vector.tensor_scalar_add(ctr[:], ctr[:], 1.0)

            # xT[e, s] from x[s, e] via PE transposes; x staged in 4 big DMAs
            xT = big.tile([128, NEC, S], f32r, tag="xT")
            for g in range(4):
                xload = scr.tile([128, 4, E], f32, tag="xload")
                nc.sync.dma_start(
                    xload[:],
                    x_d.ap()[g * 512:(g + 1) * 512, :].rearrange("(a p) e -> p a e", p=128))
                for a in range(4):
                    st = g * 4 + a
                    for ec in range(NEC):
                        tp = ps.tile([128, 128], f32, tag="mmps")
                        nc.tensor.transpose(tp[:], xload[:, a, ec * 128:(ec + 1) * 128], ident[:])
                        nc.vector.tensor_copy(xT[:, ec, st * 128:(st + 1) * 128], tp[:])

            for h in range(HLOC):
                wq_t = wts.tile([128, NEC, D], f32r, tag="wq")
                wk_t = wts.tile([128, NEC, D], f32r, tag="wk")
                wv_t = wts.tile([128, NEC, D], f32r, tag="wv")
                wo_t = wts.tile([128, NDC, E], f32r, tag="wo")
                for ec in range(NEC):
                    nc.sync.dma_start(wq_t[:, ec, :], wq_d.ap()[h, ec * 128:(ec + 1) * 128, :].bitcast(f32r))
                    nc.sync.dma_start(wk_t[:, ec, :], wk_d.ap()[h, ec * 128:(ec + 1) * 128, :].bitcast(f32r))
                    nc.sync.dma_start(wv_t[:, ec, :], wv_d.ap()[h, ec * 128:(ec + 1) * 128, :].bitcast(f32r))
                for dc in range(NDC):
                    nc.sync.dma_start(wo_t[:, dc, :], wo_d.ap()[h, dc * 128:(dc + 1) * 128, :].bitcast(f32r))

                # KT[d, k] = Wk.T @ xT
                KT = big.tile([128, NDC, S], f32r, tag="KT")
                for dc in range(NDC):
                    for kt in range(S // KT_TILE):
                        kps = ps.tile([128, KT_TILE], f32, tag="mmps")
                        for ec in range(NEC):
                            nc.tensor.matmul(
                                kps[:],
                                wk_t[:, ec, dc * 128:(dc + 1) * 128],
                                xT[:, ec, kt * KT_TILE:(kt + 1) * KT_TILE],
                                start=(ec == 0), stop=(ec == NEC - 1))
                        nc.vector.tensor_copy(KT[:, dc, kt * KT_TILE:(kt + 1) * KT_TILE], kps[:])

                # V[s, d] = x @ Wv  (bf16)
                V = big.tile([128, NKC, D], bf16, tag="V")
                for sc in range(NSC):
                    vps = ps.tile([128, D], f32, tag="mmps")
                    for ec in range(NEC):
                        nc.tensor.matmul(
                            vps[:],
                            xT[:, ec, sc * 128:(sc + 1) * 128],
                            wv_t[:, ec, :],
                            start=(ec == 0), stop=(ec == NEC - 1))
                    nc.vector.tensor_copy(V[:, sc, :], vps[:])

                def emit_tail(st_):
                    # transposes + outproj of a finished q tile (PE work that
                    # depends on the DVE/ACT combine chain); emitted after the
                    # next tile's head so PE never stalls on that chain.
                    q0_, osbs_, rr_ = st_
                    for qc in range(NQC):
                        qq = q0_ + qc * 128
                        ot_t = otp.tile([128, NDC, 128], f32r, tag="ot", name=f"ot{qc}")
                        for dc in range(NDC):
                            tp = ps.tile([128, 128], f32, tag="mmps", name="tp2")
                            nc.tensor.transpose(tp[:], osbs_[qc][:, dc * 128:(dc + 1) * 128], ident[:])
                            nc.vector.tensor_copy(ot_t[:, dc, :], tp[:])
                        out_ps = ps.tile([128, E], f32, tag="mmps", name="out_ps")
                        for dc in range(NDC):
                            nc.tensor.matmul(
                                out_ps[:], ot_t[:, dc, :], wo_t[:, dc, :],
                                start=(dc == 0), stop=(dc == NDC - 1))
                        out_sb = scr.tile([128, E], f32, tag="outsb")
                        nc.vector.tensor_scalar_mul(out_sb[:], out_ps[:], rr_[:, qc:qc + 1])
                        nc.sync.dma_start(out_d.ap()[h, qq:qq + 128, :], out_sb[:])

                pending = None
                for t in range(NQT):
                    q0 = t * QT
                    kcs = [kc for kc in range(NKC) if status[t][kc] != SKIP]
                    nk = len(kcs)

                    # QT[d, q] for this q tile
                    qt_t = qtp.tile([128, NDC, QT], f32r, tag="qt")
                    for dc in range(NDC):
                        qps = ps.tile([128, QT], f32, tag="mmps")
                        for ec in range(NEC):
                            nc.tensor.matmul(
                                qps[:],
                                wq_t[:, ec, dc * 128:(dc + 1) * 128],
                                xT[:, ec, q0:q0 + QT],
                                start=(ec == 0), stop=(ec == NEC - 1))
                        nc.vector.tensor_copy(qt_t[:, dc, :], qps[:])

                    E1 = epool.tile([128, NKC, QT], bf16, tag="E1")
                    E2 = epool.tile([128, NKC, QT], bf16, tag="E2")
                    sums_ps = pss.tile([128, 2 * NQC], f32, tag="sums")
                    o_ps = [[None] * NQC, [None] * NQC]
                    for half in (0, 1):
                        for qc in range(NQC):
                            o_ps[half][qc] = pso.tile([128, D], f32, tag="ops", name=f"ops{half}{qc}")

                    # fused k-chunk-major: scores -> exp(+mask); AV + sums run
                    # one k chunk behind so PE never waits on the exp latency
                    def emit_av(i, kc):
                        for half in (0, 1):
                            Et = E1 if half == 0 else E2
                            for qc in range(NQC):
                                lhsT = Et[:, kc, qc * 128:(qc + 1) * 128]
                                nc.tensor.matmul(
                                    o_ps[half][qc][:], lhsT, V[:, kc, :],
                                    start=(i == 0), stop=(i == nk - 1))
                                nc.tensor.matmul(
                                    sums_ps[:, half * NQC + qc:half * NQC + qc + 1],
                                    lhsT, ones_bf[:],
                                    start=(i == 0 and half == 0 and qc == 0),
                                    stop=(i == nk - 1 and half == 1 and qc == NQC - 1))

                    for i, kc in enumerate(kcs):
                        for half in (0, 1):
                            sps = ps.tile([128, QT], f32, tag="mmps")
                            for j in (0, 1):
                                dc = half * 2 + j
                                nc.tensor.matmul(
                                    sps[:],
                                    KT[:, dc, kc * 128:(kc + 1) * 128],
                                    qt_t[:, dc, :],
                                    start=(j == 0), stop=(j == 1))
                            Et = E1 if half == 0 else E2
                            nc.scalar.activation(Et[:, kc, :], sps[:], AF.Exp, scale=SCALE)
                            pat = status[t][kc]
                            if pat >= 0:
                                nc.vector.tensor_tensor(
                                    out=Et[:, kc, :], in0=Et[:, kc, :],
                                    in1=keeps_t[:, pat, :], op=ALU.mult)
                        if i > 0:
                            emit_av(i - 1, kcs[i - 1])
                    emit_av(nk - 1, kcs[nk - 1])

                    if pending is not None:
                        emit_tail(pending)
                        pending = None

                    # combine + rms per q chunk (DVE/ACT; overlaps next head)
                    rec = scr.tile([128, 2 * NQC], f32, tag="rec")
                    nc.vector.reciprocal(rec[:], sums_ps[:])
                    nc.vector.tensor_tensor(
                        out=rec[:, NQC:2 * NQC],
                        in0=rec[:, NQC:2 * NQC],
                        in1=lam_t[:, h:h + 1].to_broadcast([128, NQC]),
                        op=ALU.mult)
                    ms = scr.tile([128, NQC], f32, tag="ms")
                    osbs = []
                    for qc in range(NQC):
                        osb = scr.tile([128, D], f32, tag=f"osb{qc}")
                        nc.vector.tensor_scalar_mul(osb[:], o_ps[0][qc][:], rec[:, qc:qc + 1])
                        osb2 = scr.tile([128, D], f32, tag=f"osb2{qc}")
                        nc.vector.scalar_tensor_tensor(
                            out=osb2[:], in0=o_ps[1][qc][:],
                            scalar=rec[:, NQC + qc:NQC + qc + 1],
                            in1=osb[:], op0=ALU.mult, op1=ALU.add)
                        osq = scr.tile([128, D], f32, tag="osq")
                        nc.scalar.activation(
                            osq[:], osb2[:], AF.Square,
                            scale=float(1.0 / np.sqrt(D)), accum_out=ms[:, qc:qc + 1])
                        osbs.append(osb2)
                    lnm = scr.tile([128, NQC], f32, tag="lnm")
                    nc.scalar.activation(lnm[:], ms[:], AF.Ln, bias=eps_t[:])
                    rr = scr.tile([128, NQC], f32, tag="rr")
                    nc.scalar.activation(rr[:], lnm[:], AF.Exp, scale=-0.5)
                    pending = (q0, osbs, rr)
                if pending is not None:
                    emit_tail(pending)
                    pending = None

            if rep_ctx is not None:
                rep_ctx.__exit__(None, None, None)
                nc.sync.dma_start(iters_d.ap()[:], ctr[:])

    nc.compile()
    return nc


_CACHE = {}


def _get_program(mask, repeat=1):
    key = (mask.tobytes(), repeat)
    if key not in _CACHE:
        status, pats = _analyze_mask(mask)
        nc = _build(status, len(pats), repeat=repeat)
        _CACHE[key] = (nc, pats)
    return _CACHE[key]


def make_in_maps(x, mask, Wq, bq, Wk, bk, Wv, bv, lq1, lk1, lq2, lk2,
                 lam_init_p, rms_w, Wo, bo, repeat=1):
    x = np.asarray(x, np.float32)
    mask = np.asarray(mask, bool)
    Wq = np.asarray(Wq, np.float32)
    Wk = np.asarray(Wk, np.float32)
    Wv = np.asarray(Wv, np.float32)
    Wo = np.asarray(Wo, np.float32)
    for b_ in (bq, bk, bv):
        assert np.abs(np.asarray(b_)).max() == 0.0, "nonzero qkv bias unsupported"
    lam = (np.exp((np.asarray(lq1, np.float32) * np.asarray(lk1, np.float32)).sum(-1))
           - np.exp((np.asarray(lq2, np.float32) * np.asarray(lk2, np.float32)).sum(-1))
           + np.asarray(lam_init_p, np.float32))  # [H]
    woF = Wo.reshape(H, D, E) * ((1.0 - LAMBDA_INIT) * np.asarray(rms_w, np.float32))[:, :, None]

    nc, pats = _get_program(mask, repeat=repeat)
    if pats:
        keeps = np.stack(pats).astype(ml_dtypes.bfloat16)
    else:
        keeps = np.zeros((1, 128, QT), ml_dtypes.bfloat16)

    in_maps = []
    for c in range(NCORES):
        b = c // 4
        h0 = HLOC * (c % 4)
        lamneg = np.repeat((-lam[h0:h0 + HLOC]).astype(np.float32)[:, None, None], 128, axis=1)
        in_maps.append({
            "x": np.ascontiguousarray(x[b]),
            "wq": np.ascontiguousarray(Wq[h0:h0 + HLOC]),
            "wk": np.ascontiguousarray(Wk[h0:h0 + HLOC]),
            "wv": np.ascontiguousarray(Wv[h0:h0 + HLOC]),
            "wo": np.ascontiguousarray(woF[h0:h0 + HLOC]),
            "lamneg": np.ascontiguousarray(lamneg),
            "keeps": keeps,
        })
    return nc, in_maps


def gather(results, bo):
    out = np.zeros((B, S, E), np.float32)
    for c in range(NCORES):
        out[c // 4] += results[c]["out"].sum(axis=0)
    out += np.asarray(bo, np.float32)[None, None, :]
    return out


def kernel(**inputs):
    nc, in_maps = make_in_maps(**inputs)
    res = run_bass_kernel_spmd(nc, in_maps, core_ids=list(range(NCORES)))
    return gather(res.results, inputs["bo"])
